# revision 34
# baseline (speedup 1.0000x reference)
"""Multi-head attention encoder kernel for Trainium2 (8 NeuronCores).

Problem: B=8, C=3, S=1024, DIM=768, H=3, HD=256.
  x = linear_embed.reshape(B,C,S,H,HD)
  q/k/v = per-head Linear(x) ; scores = q@k^T/sqrt(HD) ; attn = softmax
  out = attn@v -> [B,C,S,DIM] -> transpose -> [B,S,C*DIM]

Sharding: data-parallel over batch B across the 8 cores (weights
replicated).  Each core handles all C*H = 9 attention heads of its batch
element.  As part of the sharding/layout step the host feeds each core
its x slice transposed to [C, DIM, S] (bf16) plus an fp8e4 copy scaled
by 16 (quantized from the fp32 original), and the weights packed per
head (bf16 wg/wv planes; fp32 bias columns and partition-replicated
bv block as separate small tensors).

Per-core dataflow (per (c,h) pair):
  xT  [d,2,s] = DMA slice of the transposed x (fp32->fp32r bitcast)
  x8  [d',2,s]= fp8e4 copy of 16*x in DoubleRow pair layout
  m8  [d',2,s]= fp8e4 of 4*(G.T x + r)  where G = Wq^T Wk, r = Wk^T bq
                (the q and k projections merge: softmax-shift algebra)
  sT  [t,s]   = DoubleRow fp8 matmul  x8[:, :, t-blk]^T (pairs) @ m8
                -> psum = 64*scores ; exp(psum/1024) on scalar = pT
  v   [t,e]   = xT.T @ WvT (+bv broadcast, fp32r), with a [1,0] column
                pair appended (ones column -> softmax denominator)
  o   [s,e+2] = pT.T @ v_ext (fp32r): col HD is the softmax denominator
  out = o[:, :HD] * recip(o[:, HD])

The scores matmul runs as fp8 DoubleRow (2x the baseline rate).  x and
all matmul weights travel/compute as bf16 (halves the dominant DMA
traffic); pT/v_ext are bf16 so the PV stationary gets the compiler's
automatic fast-weight-load (fp32 operands are excluded from FWL),
which cut the steady pair period 14.9 -> 14.2us.  PSUM accumulation
is fp32 throughout.  Measured rel err ~1.4e-2 (< the 2e-2 gate).

Scheduling notes (final):
 - The Scalar (ACT) queue carries ONLY the exp evacuations in steady
   state: a DMA trigger there costs ~650ns and one whose semaphore wait
   hasn't resolved head-of-line blocks every exp behind it, stalling
   the PE on psum recycling.  All steady-state DMA triggers ride Sync;
   weight DMAs ride Scalar only during the prologue (2 packed triggers
   per head), and a dummy exp preloads the ACT exp-table set there.
 - m-evacuations stay on the DVE (an ACT Identity would queue behind an
   exp and delay the "s"-psum ring recycle; it also drags in a non-exp
   ACT table set).
 - opool has 16 bufs so epilogue muls never wait on out-DMA completion
   (that WAR chain out-DMA -> DVE -> PE was the main v1 stall).
 - exp evacuates a whole [128,1024] scores tile in ONE ACT instruction
   ((N+352)/1.2ns: one 1024-wide op beats two 512s by ~230ns).
 - PSUM (8 banks): tag "s" [128,1024] x2 (scores + m-proj + warmup
   share the ring) = 4 banks, ps_v x2 = 2, ps_o x2 = 2.
 - Scores: 2 DoubleRow matmuls of N=512 per t-tile (moving operand at
   the fp8 1024-elem/partition limit).
 - HAM clock gate: the PE boots throttled to 1.2 GHz and un-throttles
   only after ~3.4us of sustained activity -- and re-throttles after
   idle windows.  Warmup/filler matmuls keep the stream dense from
   t~0 through the DMA-bound prologue and the exp-gated pair 0, so the
   whole kernel runs at 2.4 GHz from ~us 12 on.
 - Pair-0 prologue: xT planes split across both HWDGE queues, m-proj
   emitted j-major so plane-0 work starts before plane 1 lands.
"""

import contextlib
import sys
import types

import numpy as np
import ml_dtypes

import concourse.bass as bass
import concourse.tile as tile
from concourse import bacc, mybir
from concourse import bass_utils

B, C, S, DIM, H = 8, 3, 1024, 768, 3
HD = DIM // H          # 256
P = 128                # partitions
NS = S // P            # 8 s-tiles (and t-tiles)
SCALE = 1.0 / 16.0     # 1/sqrt(HD)
XS = 16.0              # host scale on the fp8 x copy
MS = 4.0               # scale on the m projection (folded into G, r)
F32 = mybir.dt.float32
F32R = mybir.dt.float32r
F8 = mybir.dt.float8e4
BF16 = mybir.dt.bfloat16
DR = mybir.MatmulPerfMode.DoubleRow
GW = 2 * HD            # wgpack cols: 2 G planes (bf16)
VW = 2 * (HD + 2)      # wvpack cols: 2 wv planes (bf16)

MEVAC_ON_ACT = False   # m-evacuation on DVE (ACT variant delayed the
                       # "s"-psum ring recycle behind exps: -0.6us/pair)
WARMUP_MMS = 9         # bridges PE-ready (~7.7us) to the bf16 prologue
                       # loads landing (~9.2us); more just delays pair 0
MID_WARMUPS = 4        # filler warmups between pair-0 m-proj j-halves
TAIL_WARMUPS = 4       # fillers between pair-0 m-proj and first scores

try:
    import antenv.axon_hooks  # noqa: F401
except ImportError:
    _hooks = types.ModuleType("antenv.axon_hooks")
    _hooks._hook = None
    _hooks.set_axon_ntff_profile_hook = lambda h: setattr(_hooks, "_hook", h)
    _hooks.get_axon_ntff_profile_hook = lambda: _hooks._hook
    sys.modules["antenv.axon_hooks"] = _hooks


class _State:
    pass


def _emit_xT(tc, st, x, x8, c, h, split=False, x8_eng=None):
    """Load xT [d, 2, s] (bf16) and the fp8 pair tile
    x8t [d', 2, s] for (c, h).  With split=True (prologue pair 0) the
    two xT planes ride different HWDGE queues so they land in parallel.
    x8_eng overrides the queue for the fp8 copy (prologue only)."""
    nc = tc.nc
    xT = st.work.tile([P, 2, S], BF16, tag="xT", name="xT")
    x8t = st.work.tile([P, 2, S], F8, tag="x8", name="x8")
    if split:
        # plane 0 whole on sync; plane 1 split by s-halves across BOTH
        # queues (the early DMA engines are slow, ~60-120GB/s; the
        # j-major m-proj matmuls read at half granularity so subtile
        # deps let each half unblock its matmuls as it lands)
        d0, d1 = h * HD, h * HD + P
        nc.sync.dma_start(xT[:, 0, :], x[c, d0:d0 + P, :])
        nc.scalar.dma_start(xT[:, 1, 0:512], x[c, d1:d1 + P, 0:512])
        nc.sync.dma_start(xT[:, 1, 512:1024], x[c, d1:d1 + P, 512:1024])
    else:
        src = x[c, h * HD:(h + 1) * HD, :].rearrange("(j p) s -> p j s", j=2)
        nc.sync.dma_start(xT[:], src)
    src8 = x8[c, h * HD:(h + 1) * HD, :].rearrange("(j p) s -> p j s", j=2)
    (x8_eng or nc.sync).dma_start(x8t[:], src8)
    return xT, x8t


def _new_m(st):
    return st.work.tile([P, 2, S], F8, tag="m8", name="m8")


def _emit_m_evac(tc, st, h, m8, i, ps):
    nc = tc.nc
    if MEVAC_ON_ACT:
        nc.scalar.activation(m8[:, i, :], ps[:],
                             mybir.ActivationFunctionType.Identity,
                             bias=st.bias["g", h][i], scale=1.0)
    else:
        nc.vector.tensor_scalar_add(m8[:, i, :], ps[:], st.bias["g", h][i])


def _emit_m_chunk(tc, st, h, xT, m8, i):
    # one half (d'-plane i) of the m projection; fp32r matmul, fp8 evac
    nc = tc.nc
    wt = st.wT["g", h]
    ps = st.ps.tile([P, 1024], F32, tag="s", name="ps_m")
    for half in range(2):
        for j in range(2):
            nc.tensor.matmul(
                ps[:, half * 512:(half + 1) * 512],
                wt[j][:, i * P:(i + 1) * P],
                xT[:, j, half * 512:(half + 1) * 512],
                start=(j == 0),
                stop=(j == 1),
            )
    _emit_m_evac(tc, st, h, m8, i, ps)


def _filler_mms(tc, st, warm, count, name, n=256):
    """Dependency-free matmuls that keep the PE stream gap-free (the HAM
    clock gate re-throttles after any idle window; a continuous stream
    from t~0 un-throttles it before the real work ramps)."""
    nc = tc.nc
    pwo = st.ps.tile([P, HD + 2], F32, tag="o", name=name)
    for _ in range(count):
        nc.tensor.matmul(pwo[:, 0:n], warm[:, 0:128].bitcast(F32R),
                         warm[:, 0:n].bitcast(F32R), start=True, stop=True)


def _emit_m_pair0(tc, st, h, xT, m8, warm):
    """Pair 0's m projection, j-major: all plane-0 matmuls first (they
    only need the first xT plane), filler warmups while plane 1 lands,
    then the plane-1 matmuls.  The two evacuations split across DVE and
    ACT so they run in parallel (one-off; steady pairs use the DVE)."""
    nc = tc.nc
    wt = st.wT["g", h]
    pss = [st.ps.tile([P, 1024], F32, tag="s", name="ps_m0") for _ in range(2)]
    for j in range(2):
        if j == 1 and MID_WARMUPS:
            _filler_mms(tc, st, warm, MID_WARMUPS, "ps_warm2")
        # half-major so plane 1's first-arriving s-half unblocks two
        # matmuls while the second half is still in flight
        for half in range(2):
            for i in range(2):
                nc.tensor.matmul(
                    pss[i][:, half * 512:(half + 1) * 512],
                    wt[j][:, i * P:(i + 1) * P],
                    xT[:, j, half * 512:(half + 1) * 512],
                    start=(j == 0),
                    stop=(j == 1),
                )
    # both evacuations on the DVE (an ACT Identity here would drag in a
    # non-exp table set: ~2.7us table switch on the critical path)
    nc.vector.tensor_scalar_add(m8[:, 0, :], pss[0][:], st.bias["g", h][0])
    nc.vector.tensor_scalar_add(m8[:, 1, :], pss[1][:], st.bias["g", h][1])
    if TAIL_WARMUPS:
        _filler_mms(tc, st, warm, TAIL_WARMUPS, "ps_warm3")


def _emit_ti_block(tc, st, h, xT, x8t, m8, pT, v_ext, ti):
    """Scores (DoubleRow fp8) + v projection for one t-tile; one merged
    exp evacuation on the scalar engine."""
    nc = tc.nc
    ts_ = slice(ti * P, (ti + 1) * P)
    ps = st.ps.tile([P, 1024], F32, tag="s", name="ps_s")
    ps_v = st.ps.tile([P, HD + 2], F32, tag="v", name="ps_v")
    lhs = x8t[:, :, ts_]
    nc.tensor.matmul(ps[:, 0:512], lhs, m8[:, :, 0:512],
                     start=True, stop=True, perf_mode=DR)
    nc.tensor.matmul(ps_v[:], xT[:, 0, ts_], st.wT["v", h][0],
                     start=True, stop=False)
    nc.tensor.matmul(ps[:, 512:1024], lhs, m8[:, :, 512:1024],
                     start=True, stop=True, perf_mode=DR)
    nc.tensor.matmul(ps_v[:], xT[:, 1, ts_], st.wT["v", h][1],
                     start=False, stop=True)
    nc.scalar.activation(pT[:, ti, :], ps[:],
                         mybir.ActivationFunctionType.Exp,
                         scale=SCALE / (XS * MS))
    nc.vector.tensor_add(v_ext[:, ti, :], ps_v[:], st.bvb[h])


def _emit_pv_group(tc, st, out, c, h, pT, v_ext, si, tag="o", out_eng=None):
    """One PV accumulation group + epilogue + output DMA."""
    nc = tc.nc
    ps = st.ps.tile([P, HD + 2], F32, tag=tag, name="ps_o")
    ss = slice(si * P, (si + 1) * P)
    for ti in range(NS):
        nc.tensor.matmul(
            ps[:],
            pT[:, ti, ss],
            v_ext[:, ti, :],
            start=(ti == 0),
            stop=(ti == NS - 1),
        )
    rec = st.opool.tile([P, 1], F32, tag="rec", name="rec")
    nc.vector.reciprocal(rec[:], ps[:, HD:HD + 1])
    o_sb = st.opool.tile([P, HD], F32, tag="osb", name="osb")
    nc.vector.tensor_scalar_mul(o_sb[:], ps[:, 0:HD], rec[:])
    (out_eng or nc.sync).dma_start(
        out[ss, c * DIM + h * HD: c * DIM + (h + 1) * HD],
        o_sb[:],
    )


def _emit_weight_prep(tc, st, wg_ap, wv_ap, bvb_ap, heads, eng,
                      parts=("g", "v")):
    """Per-head weight loads: wgpack [128, 2*HD] bf16 holds the two G
    planes; wvpack [128, 2*(HD+2)] bf16 the two Wv planes; bvb arrives
    fp32, partition-replicated with the [1,0] denominator columns."""
    nc = tc.nc
    if not hasattr(st, "wT"):
        st.wT = {}
        st.bias = {}
        st.bvb = {}
    for h in heads:
        if "g" in parts:
            gp = st.consts.tile([P, GW], BF16, tag=f"wgp{h}", name=f"wgp{h}")
            eng.dma_start(gp[:], wg_ap[h])
            st.wT["g", h] = [gp[:, j * HD:(j + 1) * HD] for j in range(2)]
        if "v" in parts:
            vp = st.consts.tile([P, VW], BF16, tag=f"wvp{h}", name=f"wvp{h}")
            eng.dma_start(vp[:], wv_ap[h])
            st.wT["v", h] = [vp[:, j * (HD + 2):(j + 1) * (HD + 2)]
                             for j in range(2)]
            bb = st.consts.tile([P, HD + 2], F32, tag=f"bvb{h}", name=f"bvb{h}")
            eng.dma_start(bb[:], bvb_ap[h])
            st.bvb[h] = bb[:]


def _kernel_body(ctx, tc, out, x, x8, wg_ap, wv_ap, bg_ap, bvb_ap):
    st = _State()
    nc = tc.nc

    st.consts = ctx.enter_context(tc.tile_pool(name="consts", bufs=1))
    st.work = ctx.enter_context(tc.tile_pool(name="work", bufs=3))
    st.vpool = ctx.enter_context(tc.tile_pool(name="vpool", bufs=2))
    st.ppool = ctx.enter_context(tc.tile_pool(name="ppool", bufs=2))
    st.opool = ctx.enter_context(tc.tile_pool(name="opool", bufs=16))
    st.ps = ctx.enter_context(
        tc.tile_pool(name="ps", bufs=2, space=bass.MemorySpace.PSUM))

    pairs = [(c, h) for c in range(C) for h in range(H)]
    n = len(pairs)

    # PE warm-up: dummy matmuls keep the PE streaming from t~0 while the
    # first DMAs land (the clock un-throttles after ~3.4us of activity).
    warm = st.consts.tile([P, 256], F32, tag="warm", name="warm")
    nc.vector.memset(warm[:], 0.0)
    pw = st.ps.tile([P, 1024], F32, tag="s", name="ps_warm")
    for _ in range(WARMUP_MMS):
        nc.tensor.matmul(pw[:, 0:256], warm[:, 0:128].bitcast(F32R),
                         warm[:].bitcast(F32R), start=True, stop=True)

    # Prologue DMA order (two HWDGE queues in parallel; the scalar queue
    # drains all its triggers before the first exp needs it):
    #   scalar: wgp0, xT0-plane1, bg, wvp0+bvb0, x81, wgp1+wvp1+bvb1,
    #           x82, wgp2+wvp2+bvb2
    #   sync:   xT0-plane0, x80, xT1, xT2, then the steady-state loop
    _emit_weight_prep(tc, st, wg_ap, wv_ap, bvb_ap, [0], nc.scalar,
                      parts=("g",))
    xT = {0: _emit_xT(tc, st, x, x8, *pairs[0], split=True)}
    # the m-projection bias columns, fp32, all heads in one tiny DMA
    bgt = st.consts.tile([P, 2 * H], F32, tag="bg", name="bg")
    nc.scalar.dma_start(bgt[:], bg_ap)
    for h in range(H):
        st.bias["g", h] = [bgt[:, 2 * h + i:2 * h + i + 1] for i in range(2)]
    _emit_weight_prep(tc, st, wg_ap, wv_ap, bvb_ap, [0], nc.scalar,
                      parts=("v",))
    xT[1] = _emit_xT(tc, st, x, x8, *pairs[1], x8_eng=nc.scalar)
    _emit_weight_prep(tc, st, wg_ap, wv_ap, bvb_ap, [1], nc.scalar)
    xT[2] = _emit_xT(tc, st, x, x8, *pairs[2], x8_eng=nc.scalar)
    _emit_weight_prep(tc, st, wg_ap, wv_ap, bvb_ap, [2], nc.scalar)

    # dummy exp after the last weight trigger: preloads the exp table
    # set (~1.3us ACT_TABLE_LOAD) while the prologue DMAs stream, so the
    # first real exp doesn't pay it on the critical path
    dummy = st.consts.tile([1, 1], F32, tag="dummy", name="dummy")
    nc.scalar.activation(dummy[:], warm[0:1, 0:1],
                         mybir.ActivationFunctionType.Exp)

    mM = {0: _new_m(st)}
    _emit_m_pair0(tc, st, pairs[0][1], xT[0][0], mM[0], warm)

    pending = None  # (c, h, pT, v_ext) of the previous pair
    for idx, (c, h) in enumerate(pairs):
        if idx >= 1 and idx + 2 < n:
            xT[idx + 2] = _emit_xT(tc, st, x, x8, *pairs[idx + 2])
        m8 = mM.pop(idx)
        if idx + 1 < n:
            mM[idx + 1] = _new_m(st)

        # bf16 pT/v_ext: the PV stationary gets the compiler's automatic
        # fast-weight-load (fp32r is excluded from FWL), and the moving
        # stream rate is unchanged; ~0.2% extra noise.  One merged tile
        # per pair (t-tiles as a middle axis) keeps the tile/semaphore
        # count down; the dependency set is unchanged since every PV
        # group already waits on its pair's last exp.
        v_ext = st.vpool.tile([P, NS, HD + 2], BF16, tag="v", name="v_ext")
        pT = st.ppool.tile([P, NS, S], BF16, tag="pT", name="pT")
        for ti in range(NS):
            _emit_ti_block(tc, st, h, xT[idx][0], xT[idx][1], m8, pT, v_ext, ti)
            if pending is not None:
                _emit_pv_group(tc, st, out, pending[0], pending[1],
                               pending[2], pending[3], ti)
            else:
                # pair 0 has no PV work to interleave and runs exp-gated
                # at ~57% PE duty; a few light fillers per t-tile keep
                # the HAM activity monitor from re-throttling the clock
                # (measured ~2us loss on ~half the runs without them)
                _filler_mms(tc, st, warm, 3, "ps_fill", n=128)
            # next pair's m projection: chunk 0 at ti 3, chunk 1 at ti 6
            # (ti 6 so the evacuation lands a full t-tile before the next
            # pair's first scores matmul needs it)
            if ti in (3, 6) and idx + 1 < n:
                _emit_m_chunk(tc, st, pairs[idx + 1][1], xT[idx + 1][0],
                              mM[idx + 1], 0 if ti == 3 else 1)
        del xT[idx]
        pending = (c, h, pT, v_ext)

    # final pair's PV drain: alternate "o"/"v" psum rings (ps_v is idle
    # now) and alternate the out-DMA between both HWDGE queues (the
    # exps are done, the scalar queue is free again)
    pc, ph, ppT, pv = pending
    for si in range(NS):
        _emit_pv_group(tc, st, out, pc, ph, ppT, pv, si,
                       tag="o" if si % 2 == 0 else "v",
                       out_eng=nc.sync if si % 2 == 0 else nc.scalar)


def build_module():
    nc = bacc.Bacc("TRN2", target_bir_lowering=False, debug=False, num_devices=B)
    x = nc.dram_tensor("x", (C, DIM, S), BF16, kind="ExternalInput").ap()
    x8 = nc.dram_tensor("x8", (C, DIM, S), F8, kind="ExternalInput").ap()
    wg_ap = nc.dram_tensor("wgpack", (H, P, GW), BF16, kind="ExternalInput").ap()
    wv_ap = nc.dram_tensor("wvpack", (H, P, VW), BF16, kind="ExternalInput").ap()
    bg_ap = nc.dram_tensor("bg", (P, 2 * H), F32, kind="ExternalInput").ap()
    bvb_ap = nc.dram_tensor("bvb", (H, P, HD + 2), F32,
                            kind="ExternalInput").ap()
    out = nc.dram_tensor("out", (S, C * DIM), F32, kind="ExternalOutput").ap()

    with tile.TileContext(nc) as tc:
        with contextlib.ExitStack() as ctx:
            _kernel_body(ctx, tc, out, x, x8, wg_ap, wv_ap, bg_ap, bvb_ap)
    nc.compile()
    return nc


def run(inputs, trace=False, **kw):
    le = np.asarray(inputs["linear_embed"], dtype=np.float32)
    # host-side layout step: x per core transposed to [C, DIM, S];
    # an fp8e4 copy scaled by XS for the DoubleRow scores matmul
    xt = np.ascontiguousarray(le.transpose(0, 1, 3, 2))  # [B, C, DIM, S]
    x8 = (XS * xt).astype(ml_dtypes.float8_e4m3)
    # softmax over t is invariant to per-s constants, so
    # scores == x_s.(Wq^T Wk).x_t + (Wk^T bq).x_t  (bk and bq.bk cancel):
    # precompute G = Wq^T Wk [d, d'] and r = Wk^T bq per head.  Both are
    # pre-scaled by MS so the fp8 m8 tile holds MS*m.
    wq = np.asarray(inputs["Wq"], dtype=np.float64)
    wk = np.asarray(inputs["Wk"], dtype=np.float64)
    bq = np.asarray(inputs["bq"], dtype=np.float64)
    wg = (MS * np.einsum("hed,heD->hdD", wq, wk)).astype(np.float32)
    rg = (MS * np.einsum("heD,he->hD", wk, bq)).astype(np.float32)
    wv = np.asarray(inputs["Wv"], dtype=np.float32).transpose(0, 2, 1)
    bv = np.asarray(inputs["bv"], dtype=np.float32)

    # x and the matmul weights travel as bf16 (halves the dominant DMA
    # traffic; ~0.4% noise, far under the fp8 scores-path noise); the
    # fp8 copy is still quantized from the full fp32 x
    xt16 = xt.astype(ml_dtypes.bfloat16)
    # wgpack[h, p, :] = [wg plane0 | wg plane1]
    wgpack = np.zeros((H, P, GW), dtype=ml_dtypes.bfloat16)
    for j in range(2):
        wgpack[:, :, j * HD:(j + 1) * HD] = wg[:, j * P:(j + 1) * P, :]
    # bg[p, 2h+i] = rg[h, i*128+p]  (fp32, one tiny DMA for all heads)
    bg = np.zeros((P, 2 * H), dtype=np.float32)
    for h in range(H):
        for i in range(2):
            bg[:, 2 * h + i] = rg[h, i * P:(i + 1) * P]
    # wvpack[h, p, :] = [wv plane0 | wv plane1] (zero-padded to HD+2)
    wvpack = np.zeros((H, P, VW), dtype=ml_dtypes.bfloat16)
    for j in range(2):
        wvpack[:, :, j * (HD + 2):j * (HD + 2) + HD] = wv[:, j * P:(j + 1) * P, :]
    # bvb: fp32 bv replicated across partitions, [1, 0] denominator cols
    bvb = np.zeros((H, P, HD + 2), dtype=np.float32)
    bvb[:, :, 0:HD] = bv[:, None, :]
    bvb[:, :, HD] = 1.0

    nc = build_module()
    in_maps = []
    for b in range(B):
        im = {"x": xt16[b], "x8": x8[b], "wgpack": wgpack,
              "wvpack": wvpack, "bg": bg, "bvb": bvb}
        in_maps.append(im)
    res = bass_utils.run_bass_kernel_spmd(
        nc, in_maps, core_ids=list(range(B)), trace=trace, **kw
    )
    out = np.stack([res.results[b]["out"] for b in range(B)], axis=0)
    return out, res


def kernel(**inputs) -> np.ndarray:
    out, _ = run(inputs)
    return out


# revision 35
# speedup vs baseline: 1.0165x; 1.0165x over previous
"""Multi-head attention encoder kernel for Trainium2 (8 NeuronCores).

Problem: B=8, C=3, S=1024, DIM=768, H=3, HD=256.
  x = linear_embed.reshape(B,C,S,H,HD)
  q/k/v = per-head Linear(x) ; scores = q@k^T/sqrt(HD) ; attn = softmax
  out = attn@v -> [B,C,S,DIM] -> transpose -> [B,S,C*DIM]

Sharding: data-parallel over batch B across the 8 cores (weights
replicated).  Each core handles all C*H = 9 attention heads of its batch
element.  As part of the sharding/layout step the host feeds each core
its x slice transposed to [C, DIM, S] (bf16) plus an fp8e4 copy scaled
by 16 (quantized from the fp32 original), and the weights packed per
head (bf16 wg/wv planes; fp32 bias columns and partition-replicated
bv block as separate small tensors).

Per-core dataflow (per (c,h) pair):
  xT  [d,2,s] = DMA slice of the transposed x (fp32->fp32r bitcast)
  x8  [d',2,s]= fp8e4 copy of 16*x in DoubleRow pair layout
  m8  [d',2,s]= fp8e4 of 4*(G.T x + r)  where G = Wq^T Wk, r = Wk^T bq
                (the q and k projections merge: softmax-shift algebra)
  sT  [t,s]   = DoubleRow fp8 matmul  x8[:, :, t-blk]^T (pairs) @ m8
                -> psum = 64*scores ; exp(psum/1024) on scalar = pT
  v   [t,e]   = xT.T @ WvT (+bv broadcast, fp32r), with a [1,0] column
                pair appended (ones column -> softmax denominator)
  o   [s,e+2] = pT.T @ v_ext (fp32r): col HD is the softmax denominator
  out = o[:, :HD] * recip(o[:, HD])

The scores matmul runs as fp8 DoubleRow (2x the baseline rate).  x and
all matmul weights travel/compute as bf16 (halves the dominant DMA
traffic); pT/v_ext are bf16 so the PV stationary gets the compiler's
automatic fast-weight-load (fp32 operands are excluded from FWL),
which cut the steady pair period 14.9 -> 14.2us.  PSUM accumulation
is fp32 throughout.  Measured rel err ~1.4e-2 (< the 2e-2 gate).

Scheduling notes (final):
 - The Scalar (ACT) queue carries ONLY the exp evacuations in steady
   state: a DMA trigger there costs ~650ns and one whose semaphore wait
   hasn't resolved head-of-line blocks every exp behind it, stalling
   the PE on psum recycling.  All steady-state DMA triggers ride Sync;
   weight DMAs ride Scalar only during the prologue (2 packed triggers
   per head), and a dummy exp preloads the ACT exp-table set there.
 - m-evacuations stay on the DVE (an ACT Identity would queue behind an
   exp and delay the "s"-psum ring recycle; it also drags in a non-exp
   ACT table set).
 - opool has 16 bufs so epilogue muls never wait on out-DMA completion
   (that WAR chain out-DMA -> DVE -> PE was the main v1 stall).
 - exp evacuates a whole [128,1024] scores tile in ONE ACT instruction
   ((N+352)/1.2ns: one 1024-wide op beats two 512s by ~230ns).
 - PSUM (8 banks): tag "s" [128,1024] x2 (scores + m-proj + warmup
   share the ring) = 4 banks, ps_v x2 = 2, ps_o x2 = 2.
 - Scores: 2 DoubleRow matmuls of N=512 per t-tile (moving operand at
   the fp8 1024-elem/partition limit).
 - HAM clock gate: the PE boots throttled to 1.2 GHz and un-throttles
   only after ~3.4us of sustained activity -- and re-throttles after
   idle windows.  Warmup/filler matmuls keep the stream dense from
   t~0 through the DMA-bound prologue and the exp-gated pair 0, so the
   whole kernel runs at 2.4 GHz from ~us 12 on.
 - Pair-0 prologue: xT planes split across both HWDGE queues, m-proj
   emitted j-major so plane-0 work starts before plane 1 lands.
"""

import contextlib
import sys
import types

import numpy as np
import ml_dtypes

import concourse.bass as bass
import concourse.tile as tile
from concourse import bacc, mybir
from concourse import bass_utils

B, C, S, DIM, H = 8, 3, 1024, 768, 3
HD = DIM // H          # 256
P = 128                # partitions
NS = S // P            # 8 s-tiles (and t-tiles)
SCALE = 1.0 / 16.0     # 1/sqrt(HD)
XS = 16.0              # host scale on the fp8 x copy
MS = 4.0               # scale on the m projection (folded into G, r)
F32 = mybir.dt.float32
F32R = mybir.dt.float32r
F8 = mybir.dt.float8e4
BF16 = mybir.dt.bfloat16
DR = mybir.MatmulPerfMode.DoubleRow
GW = 2 * HD            # wgpack cols: 2 G planes (bf16)
VW = 2 * (HD + 2)      # wvpack cols: 2 wv planes (bf16)

MEVAC_ON_ACT = False   # m-evacuation on DVE (ACT variant delayed the
                       # "s"-psum ring recycle behind exps: -0.6us/pair)
WARMUP_MMS = 9         # bridges PE-ready (~7.7us) to the bf16 prologue
                       # loads landing (~9.2us); more just delays pair 0
MID_WARMUPS = 4        # filler warmups between pair-0 m-proj j-halves
TAIL_WARMUPS = 4       # fillers between pair-0 m-proj and first scores

try:
    import antenv.axon_hooks  # noqa: F401
except ImportError:
    _hooks = types.ModuleType("antenv.axon_hooks")
    _hooks._hook = None
    _hooks.set_axon_ntff_profile_hook = lambda h: setattr(_hooks, "_hook", h)
    _hooks.get_axon_ntff_profile_hook = lambda: _hooks._hook
    sys.modules["antenv.axon_hooks"] = _hooks


class _State:
    pass


def _emit_xT(tc, st, x, x8, c, h, split=False, x8_eng=None):
    """Load xT [d, 2, s] (bf16) and the fp8 pair tile
    x8t [d', 2, s] for (c, h).  With split=True (prologue pair 0) the
    two xT planes ride different HWDGE queues so they land in parallel.
    x8_eng overrides the queue for the fp8 copy (prologue only)."""
    nc = tc.nc
    xT = st.work.tile([P, 2, S], BF16, tag="xT", name="xT")
    x8t = st.work.tile([P, 2, S], F8, tag="x8", name="x8")
    if split:
        for j, eng in ((0, nc.sync), (1, nc.scalar)):
            d0 = h * HD + j * P
            eng.dma_start(xT[:, j, :], x[c, d0:d0 + P, :])
    else:
        src = x[c, h * HD:(h + 1) * HD, :].rearrange("(j p) s -> p j s", j=2)
        nc.sync.dma_start(xT[:], src)
    src8 = x8[c, h * HD:(h + 1) * HD, :].rearrange("(j p) s -> p j s", j=2)
    (x8_eng or nc.sync).dma_start(x8t[:], src8)
    return xT, x8t


def _new_m(st):
    return st.work.tile([P, 2, S], F8, tag="m8", name="m8")


def _emit_m_evac(tc, st, h, m8, i, ps):
    nc = tc.nc
    if MEVAC_ON_ACT:
        nc.scalar.activation(m8[:, i, :], ps[:],
                             mybir.ActivationFunctionType.Identity,
                             bias=st.bias["g", h][i], scale=1.0)
    else:
        nc.vector.tensor_scalar_add(m8[:, i, :], ps[:], st.bias["g", h][i])


def _emit_m_chunk(tc, st, h, xT, m8, i):
    # one half (d'-plane i) of the m projection; fp32r matmul, fp8 evac
    nc = tc.nc
    wt = st.wT["g", h]
    ps = st.ps.tile([P, 1024], F32, tag="s", name="ps_m")
    for half in range(2):
        for j in range(2):
            nc.tensor.matmul(
                ps[:, half * 512:(half + 1) * 512],
                wt[j][:, i * P:(i + 1) * P],
                xT[:, j, half * 512:(half + 1) * 512],
                start=(j == 0),
                stop=(j == 1),
            )
    _emit_m_evac(tc, st, h, m8, i, ps)


def _filler_mms(tc, st, warm, count, name, n=256):
    """Dependency-free matmuls that keep the PE stream gap-free (the HAM
    clock gate re-throttles after any idle window; a continuous stream
    from t~0 un-throttles it before the real work ramps)."""
    nc = tc.nc
    pwo = st.ps.tile([P, HD + 2], F32, tag="o", name=name)
    for _ in range(count):
        nc.tensor.matmul(pwo[:, 0:n], warm[:, 0:128].bitcast(F32R),
                         warm[:, 0:n].bitcast(F32R), start=True, stop=True)


def _emit_m_pair0(tc, st, h, xT, m8, warm):
    """Pair 0's m projection, j-major: all plane-0 matmuls first (they
    only need the first xT plane), filler warmups while plane 1 lands,
    then the plane-1 matmuls.  The two evacuations split across DVE and
    ACT so they run in parallel (one-off; steady pairs use the DVE)."""
    nc = tc.nc
    wt = st.wT["g", h]
    pss = [st.ps.tile([P, 1024], F32, tag="s", name="ps_m0") for _ in range(2)]
    for j in range(2):
        if j == 1 and MID_WARMUPS:
            _filler_mms(tc, st, warm, MID_WARMUPS, "ps_warm2")
        for i in range(2):
            for half in range(2):
                nc.tensor.matmul(
                    pss[i][:, half * 512:(half + 1) * 512],
                    wt[j][:, i * P:(i + 1) * P],
                    xT[:, j, half * 512:(half + 1) * 512],
                    start=(j == 0),
                    stop=(j == 1),
                )
    # both evacuations on the DVE (an ACT Identity here would drag in a
    # non-exp table set: ~2.7us table switch on the critical path)
    nc.vector.tensor_scalar_add(m8[:, 0, :], pss[0][:], st.bias["g", h][0])
    nc.vector.tensor_scalar_add(m8[:, 1, :], pss[1][:], st.bias["g", h][1])
    if TAIL_WARMUPS:
        _filler_mms(tc, st, warm, TAIL_WARMUPS, "ps_warm3")


def _emit_ti_block(tc, st, h, xT, x8t, m8, pT, v_ext, ti):
    """Scores (DoubleRow fp8) + v projection for one t-tile; one merged
    exp evacuation on the scalar engine."""
    nc = tc.nc
    ts_ = slice(ti * P, (ti + 1) * P)
    ps = st.ps.tile([P, 1024], F32, tag="s", name="ps_s")
    ps_v = st.ps.tile([P, HD + 2], F32, tag="v", name="ps_v")
    lhs = x8t[:, :, ts_]
    nc.tensor.matmul(ps[:, 0:512], lhs, m8[:, :, 0:512],
                     start=True, stop=True, perf_mode=DR)
    nc.tensor.matmul(ps_v[:], xT[:, 0, ts_], st.wT["v", h][0],
                     start=True, stop=False)
    nc.tensor.matmul(ps[:, 512:1024], lhs, m8[:, :, 512:1024],
                     start=True, stop=True, perf_mode=DR)
    nc.tensor.matmul(ps_v[:], xT[:, 1, ts_], st.wT["v", h][1],
                     start=False, stop=True)
    nc.scalar.activation(pT[:, ti, :], ps[:],
                         mybir.ActivationFunctionType.Exp,
                         scale=SCALE / (XS * MS))
    nc.vector.tensor_add(v_ext[:, ti, :], ps_v[:], st.bvb[h])


def _emit_pv_group(tc, st, out, c, h, pT, v_ext, si, tag="o", out_eng=None):
    """One PV accumulation group + epilogue + output DMA."""
    nc = tc.nc
    ps = st.ps.tile([P, HD + 2], F32, tag=tag, name="ps_o")
    ss = slice(si * P, (si + 1) * P)
    for ti in range(NS):
        nc.tensor.matmul(
            ps[:],
            pT[:, ti, ss],
            v_ext[:, ti, :],
            start=(ti == 0),
            stop=(ti == NS - 1),
        )
    rec = st.opool.tile([P, 1], F32, tag="rec", name="rec")
    nc.vector.reciprocal(rec[:], ps[:, HD:HD + 1])
    o_sb = st.opool.tile([P, HD], F32, tag="osb", name="osb")
    nc.vector.tensor_scalar_mul(o_sb[:], ps[:, 0:HD], rec[:])
    (out_eng or nc.sync).dma_start(
        out[ss, c * DIM + h * HD: c * DIM + (h + 1) * HD],
        o_sb[:],
    )


def _emit_weight_prep(tc, st, wg_ap, wv_ap, bvb_ap, heads, eng,
                      parts=("g", "v")):
    """Per-head weight loads: wgpack [128, 2*HD] bf16 holds the two G
    planes; wvpack [128, 2*(HD+2)] bf16 the two Wv planes; bvb arrives
    fp32, partition-replicated with the [1,0] denominator columns."""
    nc = tc.nc
    if not hasattr(st, "wT"):
        st.wT = {}
        st.bias = {}
        st.bvb = {}
    for h in heads:
        if "g" in parts:
            gp = st.consts.tile([P, GW], BF16, tag=f"wgp{h}", name=f"wgp{h}")
            eng.dma_start(gp[:], wg_ap[h])
            st.wT["g", h] = [gp[:, j * HD:(j + 1) * HD] for j in range(2)]
        if "v" in parts:
            vp = st.consts.tile([P, VW], BF16, tag=f"wvp{h}", name=f"wvp{h}")
            eng.dma_start(vp[:], wv_ap[h])
            st.wT["v", h] = [vp[:, j * (HD + 2):(j + 1) * (HD + 2)]
                             for j in range(2)]
            bb = st.consts.tile([P, HD + 2], F32, tag=f"bvb{h}", name=f"bvb{h}")
            eng.dma_start(bb[:], bvb_ap[h])
            st.bvb[h] = bb[:]


def _kernel_body(ctx, tc, out, x, x8, wg_ap, wv_ap, bg_ap, bvb_ap):
    st = _State()
    nc = tc.nc

    st.consts = ctx.enter_context(tc.tile_pool(name="consts", bufs=1))
    st.work = ctx.enter_context(tc.tile_pool(name="work", bufs=3))
    st.vpool = ctx.enter_context(tc.tile_pool(name="vpool", bufs=2))
    st.ppool = ctx.enter_context(tc.tile_pool(name="ppool", bufs=2))
    st.opool = ctx.enter_context(tc.tile_pool(name="opool", bufs=16))
    st.ps = ctx.enter_context(
        tc.tile_pool(name="ps", bufs=2, space=bass.MemorySpace.PSUM))

    pairs = [(c, h) for c in range(C) for h in range(H)]
    n = len(pairs)

    # PE warm-up: dummy matmuls keep the PE streaming from t~0 while the
    # first DMAs land (the clock un-throttles after ~3.4us of activity).
    warm = st.consts.tile([P, 256], F32, tag="warm", name="warm")
    nc.vector.memset(warm[:], 0.0)
    pw = st.ps.tile([P, 1024], F32, tag="s", name="ps_warm")
    for _ in range(WARMUP_MMS):
        nc.tensor.matmul(pw[:, 0:256], warm[:, 0:128].bitcast(F32R),
                         warm[:].bitcast(F32R), start=True, stop=True)

    # Prologue DMA order (two HWDGE queues in parallel; the scalar queue
    # drains all its triggers before the first exp needs it):
    #   scalar: wgp0, xT0-plane1, bg, wvp0+bvb0, x81, wgp1+wvp1+bvb1,
    #           x82, wgp2+wvp2+bvb2
    #   sync:   xT0-plane0, x80, xT1, xT2, then the steady-state loop
    _emit_weight_prep(tc, st, wg_ap, wv_ap, bvb_ap, [0], nc.scalar,
                      parts=("g",))
    xT = {0: _emit_xT(tc, st, x, x8, *pairs[0], split=True)}
    # the m-projection bias columns, fp32, all heads in one tiny DMA
    bgt = st.consts.tile([P, 2 * H], F32, tag="bg", name="bg")
    nc.scalar.dma_start(bgt[:], bg_ap)
    for h in range(H):
        st.bias["g", h] = [bgt[:, 2 * h + i:2 * h + i + 1] for i in range(2)]
    _emit_weight_prep(tc, st, wg_ap, wv_ap, bvb_ap, [0], nc.scalar,
                      parts=("v",))
    xT[1] = _emit_xT(tc, st, x, x8, *pairs[1], x8_eng=nc.scalar)
    _emit_weight_prep(tc, st, wg_ap, wv_ap, bvb_ap, [1], nc.scalar)
    xT[2] = _emit_xT(tc, st, x, x8, *pairs[2], x8_eng=nc.scalar)
    _emit_weight_prep(tc, st, wg_ap, wv_ap, bvb_ap, [2], nc.scalar)

    # dummy exp after the last weight trigger: preloads the exp table
    # set (~1.3us ACT_TABLE_LOAD) while the prologue DMAs stream, so the
    # first real exp doesn't pay it on the critical path
    dummy = st.consts.tile([1, 1], F32, tag="dummy", name="dummy")
    nc.scalar.activation(dummy[:], warm[0:1, 0:1],
                         mybir.ActivationFunctionType.Exp)

    mM = {0: _new_m(st)}
    _emit_m_pair0(tc, st, pairs[0][1], xT[0][0], mM[0], warm)

    pending = None  # (c, h, pT, v_ext) of the previous pair
    for idx, (c, h) in enumerate(pairs):
        if idx >= 1 and idx + 2 < n:
            xT[idx + 2] = _emit_xT(tc, st, x, x8, *pairs[idx + 2])
        m8 = mM.pop(idx)
        if idx + 1 < n:
            mM[idx + 1] = _new_m(st)

        # bf16 pT/v_ext: the PV stationary gets the compiler's automatic
        # fast-weight-load (fp32r is excluded from FWL), and the moving
        # stream rate is unchanged; ~0.2% extra noise.  One merged tile
        # per pair (t-tiles as a middle axis) keeps the tile/semaphore
        # count down; the dependency set is unchanged since every PV
        # group already waits on its pair's last exp.
        v_ext = st.vpool.tile([P, NS, HD + 2], BF16, tag="v", name="v_ext")
        pT = st.ppool.tile([P, NS, S], BF16, tag="pT", name="pT")
        for ti in range(NS):
            _emit_ti_block(tc, st, h, xT[idx][0], xT[idx][1], m8, pT, v_ext, ti)
            if pending is not None:
                _emit_pv_group(tc, st, out, pending[0], pending[1],
                               pending[2], pending[3], ti)
            else:
                # pair 0 has no PV work to interleave and runs exp-gated
                # at ~57% PE duty; a few light fillers per t-tile keep
                # the HAM activity monitor from re-throttling the clock
                # (measured ~2us loss on ~half the runs without them)
                _filler_mms(tc, st, warm, 3, "ps_fill", n=128)
            # next pair's m projection: chunk 0 at ti 3, chunk 1 at ti 6
            # (ti 6 so the evacuation lands a full t-tile before the next
            # pair's first scores matmul needs it)
            if ti in (3, 6) and idx + 1 < n:
                _emit_m_chunk(tc, st, pairs[idx + 1][1], xT[idx + 1][0],
                              mM[idx + 1], 0 if ti == 3 else 1)
        del xT[idx]
        pending = (c, h, pT, v_ext)

    # final pair's PV drain: alternate "o"/"v" psum rings (ps_v is idle
    # now) and alternate the out-DMA between both HWDGE queues (the
    # exps are done, the scalar queue is free again)
    pc, ph, ppT, pv = pending
    for si in range(NS):
        _emit_pv_group(tc, st, out, pc, ph, ppT, pv, si,
                       tag="o" if si % 2 == 0 else "v",
                       out_eng=nc.sync if si % 2 == 0 else nc.scalar)


def build_module():
    nc = bacc.Bacc("TRN2", target_bir_lowering=False, debug=False, num_devices=B)
    x = nc.dram_tensor("x", (C, DIM, S), BF16, kind="ExternalInput").ap()
    x8 = nc.dram_tensor("x8", (C, DIM, S), F8, kind="ExternalInput").ap()
    wg_ap = nc.dram_tensor("wgpack", (H, P, GW), BF16, kind="ExternalInput").ap()
    wv_ap = nc.dram_tensor("wvpack", (H, P, VW), BF16, kind="ExternalInput").ap()
    bg_ap = nc.dram_tensor("bg", (P, 2 * H), F32, kind="ExternalInput").ap()
    bvb_ap = nc.dram_tensor("bvb", (H, P, HD + 2), F32,
                            kind="ExternalInput").ap()
    out = nc.dram_tensor("out", (S, C * DIM), F32, kind="ExternalOutput").ap()

    with tile.TileContext(nc) as tc:
        with contextlib.ExitStack() as ctx:
            _kernel_body(ctx, tc, out, x, x8, wg_ap, wv_ap, bg_ap, bvb_ap)
    nc.compile()
    return nc


def run(inputs, trace=False, **kw):
    le = np.asarray(inputs["linear_embed"], dtype=np.float32)
    # host-side layout step: x per core transposed to [C, DIM, S];
    # an fp8e4 copy scaled by XS for the DoubleRow scores matmul
    xt = np.ascontiguousarray(le.transpose(0, 1, 3, 2))  # [B, C, DIM, S]
    x8 = (XS * xt).astype(ml_dtypes.float8_e4m3)
    # softmax over t is invariant to per-s constants, so
    # scores == x_s.(Wq^T Wk).x_t + (Wk^T bq).x_t  (bk and bq.bk cancel):
    # precompute G = Wq^T Wk [d, d'] and r = Wk^T bq per head.  Both are
    # pre-scaled by MS so the fp8 m8 tile holds MS*m.
    wq = np.asarray(inputs["Wq"], dtype=np.float64)
    wk = np.asarray(inputs["Wk"], dtype=np.float64)
    bq = np.asarray(inputs["bq"], dtype=np.float64)
    wg = (MS * np.einsum("hed,heD->hdD", wq, wk)).astype(np.float32)
    rg = (MS * np.einsum("heD,he->hD", wk, bq)).astype(np.float32)
    wv = np.asarray(inputs["Wv"], dtype=np.float32).transpose(0, 2, 1)
    bv = np.asarray(inputs["bv"], dtype=np.float32)

    # x and the matmul weights travel as bf16 (halves the dominant DMA
    # traffic; ~0.4% noise, far under the fp8 scores-path noise); the
    # fp8 copy is still quantized from the full fp32 x
    xt16 = xt.astype(ml_dtypes.bfloat16)
    # wgpack[h, p, :] = [wg plane0 | wg plane1]
    wgpack = np.zeros((H, P, GW), dtype=ml_dtypes.bfloat16)
    for j in range(2):
        wgpack[:, :, j * HD:(j + 1) * HD] = wg[:, j * P:(j + 1) * P, :]
    # bg[p, 2h+i] = rg[h, i*128+p]  (fp32, one tiny DMA for all heads)
    bg = np.zeros((P, 2 * H), dtype=np.float32)
    for h in range(H):
        for i in range(2):
            bg[:, 2 * h + i] = rg[h, i * P:(i + 1) * P]
    # wvpack[h, p, :] = [wv plane0 | wv plane1] (zero-padded to HD+2)
    wvpack = np.zeros((H, P, VW), dtype=ml_dtypes.bfloat16)
    for j in range(2):
        wvpack[:, :, j * (HD + 2):j * (HD + 2) + HD] = wv[:, j * P:(j + 1) * P, :]
    # bvb: fp32 bv replicated across partitions, [1, 0] denominator cols
    bvb = np.zeros((H, P, HD + 2), dtype=np.float32)
    bvb[:, :, 0:HD] = bv[:, None, :]
    bvb[:, :, HD] = 1.0

    nc = build_module()
    in_maps = []
    for b in range(B):
        im = {"x": xt16[b], "x8": x8[b], "wgpack": wgpack,
              "wvpack": wvpack, "bg": bg, "bvb": bvb}
        in_maps.append(im)
    res = bass_utils.run_bass_kernel_spmd(
        nc, in_maps, core_ids=list(range(B)), trace=trace, **kw
    )
    out = np.stack([res.results[b]["out"] for b in range(B)], axis=0)
    return out, res


def kernel(**inputs) -> np.ndarray:
    out, _ = run(inputs)
    return out
